# revision 1
# baseline (speedup 1.0000x reference)
"""TRN2 Bass kernel for nn_DCM_50414326120808 (dense_cnn).

Computes, for x, convoluted [16, 256, 96, 96]:
  pooled = adaptive_avg_pool2d(x, 3)                         # [16,256,3,3]
  gen    = 1x1 conv (w_gen) of pooled + b_gen                # per-sample filters
  y      = conv3x3(convoluted, w_c1) + b_c1                  # [16,256,96,96]
  y      = relu(batchnorm_train(y) * gamma + beta)
  out    = depthwise 3x3 conv of y with per-(sample,channel) filters gen

Sharding: data-parallel over batch across 8 cores (2 samples each).
BN batch statistics are merged with an in-kernel AllReduce.

Device mapping:
 - conv3x3 -> 18 accumulated TensorE matmuls (9 taps x 2 input-channel
   chunks) per output tile, fp32r (tf32-like) at full PE rate.
   Output tiles are 4 rows x 96 cols = 384 positions; rhs uses 2D access
   patterns into a zero-padded [98,98] input so no halo garbage is computed.
 - b_c1 is dropped entirely: training-mode BN subtracts the per-channel
   mean, so a constant per-channel bias cancels exactly.
 - BN stats via DVE bn_stats on each conv PSUM tile + bn_aggr + AllReduce.
 - depthwise conv -> 9 accumulated matmuls with diagonal weight matrices
   diag(gen[:, tap]) built on DVE from an identity matrix.
"""

import os
import numpy as np

import concourse.bass as bass
import concourse.bacc as bacc
import concourse.tile as tile
from concourse import mybir, bass_utils

F32 = mybir.dt.float32
F32R = mybir.dt.float32r

B, C, H, W = 16, 256, 96, 96
FS = 3
BN_EPS = 1e-5
NCORES = 8
SPC = B // NCORES          # samples per core = 2
P = 128                    # partition dim
NIC = C // P               # input channel chunks = 2
NOC = C // P               # output channel chunks = 2
HP, WP = H + 2, W + 2      # padded spatial = 98
RT = 4                     # output rows per tile
NT = H // RT               # tiles per (sample, oc) = 24
GRP = 6                    # tiles per input group (24 rows)
NG = NT // GRP             # input groups = 4
N_LOCAL = float(SPC * H * W)        # elements per (channel, core)
N_TOTAL = float(B * H * W)          # elements per channel globally

_cache = {}


def _build_program():
    nc = bacc.Bacc("TRN2", target_bir_lowering=False, debug=False,
                   num_devices=NCORES)

    cp_d = nc.dram_tensor("cp", (SPC, NIC, P, HP, WP), F32R, kind="ExternalInput")
    x_d = nc.dram_tensor("xin", (SPC, NIC, P, H, W), F32, kind="ExternalInput")
    wT_d = nc.dram_tensor("wT", (NIC, P, 9 * NOC * P), F32R, kind="ExternalInput")
    wg_d = nc.dram_tensor("wgenT", (NIC, P, NOC * P), F32, kind="ExternalInput")
    bg_d = nc.dram_tensor("bgen", (NOC, P), F32, kind="ExternalInput")
    gam_d = nc.dram_tensor("gam", (NOC, P), F32, kind="ExternalInput")
    bet_d = nc.dram_tensor("bet", (NOC, P), F32, kind="ExternalInput")
    id_d = nc.dram_tensor("ident", (P, P), F32, kind="ExternalInput")
    out_d = nc.dram_tensor("out", (SPC, NOC, P, H, W), F32, kind="ExternalOutput")
    dbg_d = nc.dram_tensor("dbg", (P, 2 * NOC), F32, kind="ExternalOutput")

    with tile.TileContext(nc) as tc:
        with (
            tc.tile_pool(name="const", bufs=1) as const,
            tc.tile_pool(name="cin", bufs=4) as cinp,
            tc.tile_pool(name="xp", bufs=2) as xp,
            tc.tile_pool(name="small", bufs=1) as small,
            tc.tile_pool(name="ybn", bufs=1) as ybnp,
            tc.tile_pool(name="yld", bufs=3) as yldp,
            tc.tile_pool(name="evac", bufs=4) as evacp,
            tc.tile_pool(name="diag", bufs=2) as diagp,
            tc.tile_pool(name="ps_conv", bufs=3, space="PSUM") as ps_conv,
            tc.tile_pool(name="ps_dw", bufs=3, space="PSUM") as ps_dw,
            tc.tile_pool(name="ps_gen", bufs=1, space="PSUM") as ps_gen,
            tc.tile_pool(name="dram", bufs=1, space="DRAM") as dram,
        ):
            # ---- constants / weights ----
            w_sb = const.tile([P, NIC, 9 * NOC * P], F32R)
            for ic in range(NIC):
                nc.sync.dma_start(w_sb[:, ic, :], wT_d.ap()[ic])
            wg_sb = const.tile([P, NIC, NOC * P], F32)
            for ic in range(NIC):
                nc.sync.dma_start(wg_sb[:, ic, :], wg_d.ap()[ic])
            id_sb = const.tile([P, P], F32)
            nc.sync.dma_start(id_sb[:], id_d.ap())
            bg_sb = const.tile([P, NOC], F32)
            gam_sb = const.tile([P, NOC], F32)
            bet_sb = const.tile([P, NOC], F32)
            nc.sync.dma_start(bg_sb[:], bg_d.ap().rearrange("a p -> p a"))
            nc.sync.dma_start(gam_sb[:], gam_d.ap().rearrange("a p -> p a"))
            nc.sync.dma_start(bet_sb[:], bet_d.ap().rearrange("a p -> p a"))

            y_spill = dram.tile([SPC, NOC, P, H, W], F32)
            ar_in_d = dram.tile([P, 2 * NOC], F32)
            ar_out_d = dram.tile([P, 2 * NOC], F32)

            # ---- adaptive avg pool (sums; /1024 folded into wgenT) ----
            pooled = {}
            for s in range(SPC):
                for ic in range(NIC):
                    pt = small.tile([P, 9], F32, tag=f"pooled{s}{ic}",
                                    name=f"pooled{s}{ic}")
                    pooled[s, ic] = pt
                    for bi in range(3):
                        xblk = xp.tile([P, 32, W], F32)
                        nc.sync.dma_start(xblk[:], x_d.ap()[s, ic, :,
                                                            32 * bi:32 * bi + 32, :])
                        for bj in range(3):
                            nc.vector.reduce_sum(
                                pt[:, bi * 3 + bj:bi * 3 + bj + 1],
                                xblk[:, :, 32 * bj:32 * bj + 32],
                                axis=mybir.AxisListType.XY)

            # ---- filter generation: gen = wgenT.T @ pooled + b_gen ----
            gen = {}
            for s in range(SPC):
                for oc in range(NOC):
                    gps = ps_gen.tile([P, 9], F32, tag="gen", bufs=2, name="gps")
                    for ic in range(NIC):
                        nc.tensor.matmul(gps[:], wg_sb[:, ic, oc * P:(oc + 1) * P],
                                         pooled[s, ic][:],
                                         start=(ic == 0), stop=(ic == NIC - 1))
                    gt = small.tile([P, 9], F32, tag=f"gen{s}{oc}",
                                    name=f"gen{s}{oc}")
                    gen[s, oc] = gt
                    nc.scalar.activation(gt[:], gps[:],
                                         mybir.ActivationFunctionType.Identity,
                                         bias=bg_sb[:, oc:oc + 1])

            # ---- conv3x3 + BN stats + spill ----
            stats = small.tile([P, NOC, NT * SPC * 6], F32)
            for s in range(SPC):
                for g in range(NG):
                    cin = {}
                    for ic in range(NIC):
                        ct = cinp.tile([P, GRP * RT + 2, WP], F32R, name="cin")
                        cin[ic] = ct
                        nc.sync.dma_start(
                            ct[:], cp_d.ap()[s, ic, :,
                                             g * GRP * RT:(g + 1) * GRP * RT + 2, :])
                    for jj in range(GRP):
                        j = g * GRP + jj
                        for oc in range(NOC):
                            ps = ps_conv.tile([P, RT, W], F32, name="ps")
                            k = 0
                            for ic in range(NIC):
                                for t in range(9):
                                    dy, dx = t // 3, t % 3
                                    r0 = jj * RT + dy
                                    nc.tensor.matmul(
                                        ps[:],
                                        w_sb[:, ic, (t * NOC + oc) * P:
                                             (t * NOC + oc + 1) * P],
                                        cin[ic][:, r0:r0 + RT, dx:dx + W],
                                        start=(k == 0), stop=(k == 17))
                                    k += 1
                            idx = (s * NT + j) * 6
                            nc.vector.bn_stats(
                                stats[:, oc, idx:idx + 6],
                                ps[:].rearrange("p a b -> p (a b)"))
                            ysb = evacp.tile([P, RT, W], F32, name="ysb")
                            nc.scalar.copy(ysb[:], ps[:])
                            nc.sync.dma_start(
                                y_spill[s, oc, :, j * RT:(j + 1) * RT, :], ysb[:])

            # ---- merge stats, AllReduce, compute scale/bias ----
            ar_in = small.tile([P, 2 * NOC], F32)
            mvt = small.tile([P, NOC, 2], F32)
            tmp = small.tile([P, 4], F32)
            for oc in range(NOC):
                nc.vector.bn_aggr(mvt[:, oc, :], stats[:, oc, :])
                # sum = n * mean ; sumsq = n * (var + mean^2)
                nc.vector.tensor_scalar_mul(ar_in[:, 2 * oc:2 * oc + 1],
                                            mvt[:, oc, 0:1], N_LOCAL)
                nc.vector.tensor_mul(tmp[:, 0:1], mvt[:, oc, 0:1], mvt[:, oc, 0:1])
                nc.vector.tensor_add(tmp[:, 1:2], tmp[:, 0:1], mvt[:, oc, 1:2])
                nc.vector.tensor_scalar_mul(ar_in[:, 2 * oc + 1:2 * oc + 2],
                                            tmp[:, 1:2], N_LOCAL)
            nc.sync.dma_start(ar_in_d[:], ar_in[:])
            nc.gpsimd.collective_compute(
                "AllReduce", mybir.AluOpType.add,
                replica_groups=[list(range(NCORES))],
                ins=[ar_in_d.opt()], outs=[ar_out_d.opt()])
            ar_out = small.tile([P, 2 * NOC], F32)
            nc.sync.dma_start(ar_out[:], ar_out_d[:])
            nc.sync.dma_start(dbg_d.ap(), ar_out[:])

            scale = small.tile([P, NOC], F32)
            bias = small.tile([P, NOC], F32)
            w1 = small.tile([P, 8], F32)
            for oc in range(NOC):
                mu = w1[:, 0:1]
                veps = w1[:, 1:2]
                nc.vector.tensor_scalar_mul(mu, ar_out[:, 2 * oc:2 * oc + 1],
                                            1.0 / N_TOTAL)
                # var = sumsq/n - mu^2 ; veps = var + eps
                nc.vector.tensor_scalar_mul(w1[:, 2:3],
                                            ar_out[:, 2 * oc + 1:2 * oc + 2],
                                            1.0 / N_TOTAL)
                nc.vector.tensor_mul(w1[:, 3:4], mu, mu)
                nc.vector.tensor_sub(w1[:, 4:5], w1[:, 2:3], w1[:, 3:4])
                nc.vector.tensor_scalar_add(veps, w1[:, 4:5], BN_EPS)
                # r = rsqrt(veps): reciprocal + ACT sqrt + one Newton step
                inv = w1[:, 5:6]
                nc.vector.reciprocal(inv, veps)
                r = w1[:, 6:7]
                nc.scalar.activation(r, inv, mybir.ActivationFunctionType.Sqrt)
                # r <- 0.5 * r * (3 - veps * r^2)
                nc.vector.tensor_mul(w1[:, 7:8], r, r)
                nc.vector.tensor_mul(w1[:, 7:8], w1[:, 7:8], veps)
                nc.vector.tensor_scalar(w1[:, 7:8], w1[:, 7:8], -0.5, 1.5,
                                        op0=mybir.AluOpType.mult,
                                        op1=mybir.AluOpType.add)
                nc.vector.tensor_mul(r, r, w1[:, 7:8])
                # scale = gamma * r ; bias = beta - mu * scale
                nc.vector.tensor_mul(scale[:, oc:oc + 1], gam_sb[:, oc:oc + 1], r)
                nc.vector.tensor_mul(w1[:, 7:8], mu, scale[:, oc:oc + 1])
                nc.vector.tensor_sub(bias[:, oc:oc + 1], bet_sb[:, oc:oc + 1],
                                     w1[:, 7:8])

            # ---- BN apply + ReLU + dynamic depthwise conv ----
            for s in range(SPC):
                for oc in range(NOC):
                    dg = diagp.tile([P, 9, P], F32R, name="dg")
                    for t in range(9):
                        nc.vector.tensor_scalar_mul(dg[:, t, :], id_sb[:],
                                                    gen[s, oc][:, t:t + 1])
                    ybn = ybnp.tile([P, HP, WP], F32R, name="ybn")
                    U32 = mybir.dt.uint32
                    nc.gpsimd.memset(ybn[:, 0, :].bitcast(U32), 0)
                    nc.gpsimd.memset(ybn[:, HP - 1, :].bitcast(U32), 0)
                    # interior edge pads: (r, 97) and (r+1, 0) are flat-adjacent
                    pad_pairs = (ybn[:].rearrange("p a b -> p (a b)")
                                 [:, WP - 1:WP - 1 + H * WP]
                                 .rearrange("p (r t) -> p r t", t=WP)[:, :, 0:2])
                    nc.gpsimd.memset(pad_pairs.bitcast(U32), 0)
                    RB = 24
                    for rb in range(H // RB):
                        yld = yldp.tile([P, RB, W], F32, name="yld")
                        nc.sync.dma_start(
                            yld[:], y_spill[s, oc, :, rb * RB:(rb + 1) * RB, :])
                        nc.scalar.activation(
                            ybn[:, 1 + rb * RB:1 + (rb + 1) * RB, 1:W + 1],
                            yld[:], mybir.ActivationFunctionType.Relu,
                            bias=bias[:, oc:oc + 1], scale=scale[:, oc:oc + 1])
                    for j in range(NT):
                        pd = ps_dw.tile([P, RT, W], F32, name="pd")
                        for t in range(9):
                            dy, dx = t // 3, t % 3
                            nc.tensor.matmul(
                                pd[:], dg[:, t, :],
                                ybn[:, j * RT + dy:j * RT + dy + RT, dx:dx + W],
                                start=(t == 0), stop=(t == 8))
                        osb = evacp.tile([P, RT, W], F32, name="osb")
                        nc.vector.tensor_copy(osb[:], pd[:])
                        nc.sync.dma_start(
                            out_d.ap()[s, oc, :, j * RT:(j + 1) * RT, :], osb[:])

    nc.compile()
    return nc


def _prep_inputs(x, convoluted, w_gen, b_gen, w_c1, b_c1, gamma, beta):
    x = np.asarray(x, dtype=np.float32)
    convoluted = np.asarray(convoluted, dtype=np.float32)
    w_gen = np.asarray(w_gen, dtype=np.float32)
    b_gen = np.asarray(b_gen, dtype=np.float32)
    w_c1 = np.asarray(w_c1, dtype=np.float32)
    gamma = np.asarray(gamma, dtype=np.float32)
    beta = np.asarray(beta, dtype=np.float32)

    cp = np.zeros((B, NIC, P, HP, WP), np.float32)
    cp[:, :, :, 1:H + 1, 1:W + 1] = convoluted.reshape(B, NIC, P, H, W)
    xr = np.ascontiguousarray(x.reshape(B, NIC, P, H, W))
    # wT[ic, i, ((t*NOC)+oc)*P+o] = w_c1[oc*P+o, ic*P+i, dy, dx]
    wT = np.ascontiguousarray(
        w_c1.reshape(NOC, P, NIC, P, 9).transpose(2, 3, 4, 0, 1)
    ).reshape(NIC, P, 9 * NOC * P)
    # wgenT[ic, c, oc*P+o] = w_gen[oc*P+o, ic*P+c] / 1024  (pool mean divisor)
    wgT = np.ascontiguousarray(
        (w_gen[:, :, 0, 0] / 1024.0).reshape(NOC, P, NIC, P).transpose(2, 3, 0, 1)
    ).reshape(NIC, P, NOC * P)
    shared = {
        "wT": wT, "wgenT": wgT,
        "bgen": np.ascontiguousarray(b_gen.reshape(NOC, P)),
        "gam": np.ascontiguousarray(gamma.reshape(NOC, P)),
        "bet": np.ascontiguousarray(beta.reshape(NOC, P)),
        "ident": np.eye(P, dtype=np.float32),
    }
    in_maps = []
    for k in range(NCORES):
        m = dict(shared)
        m["cp"] = np.ascontiguousarray(cp[k * SPC:(k + 1) * SPC])
        m["xin"] = np.ascontiguousarray(xr[k * SPC:(k + 1) * SPC])
        in_maps.append(m)
    return in_maps


def _run(inputs, trace=False):
    if "nc" not in _cache:
        _cache["nc"] = _build_program()
    nc = _cache["nc"]
    in_maps = _prep_inputs(**inputs)
    res = bass_utils.run_bass_kernel_spmd(
        nc, in_maps, core_ids=list(range(NCORES)), trace=trace)
    outs = [r["out"].reshape(SPC, C, H, W) for r in res.results]
    full = np.concatenate(outs, axis=0)
    return full, res


def kernel(**inputs) -> np.ndarray:
    out, _ = _run(inputs, trace=False)
    return out



# revision 2
# speedup vs baseline: 1.1396x; 1.1396x over previous
"""TRN2 Bass kernel for nn_DCM_50414326120808 (dense_cnn).

Computes, for x, convoluted [16, 256, 96, 96]:
  pooled = adaptive_avg_pool2d(x, 3)                         # [16,256,3,3]
  gen    = 1x1 conv (w_gen) of pooled + b_gen                # per-sample filters
  y      = conv3x3(convoluted, w_c1) + b_c1                  # [16,256,96,96]
  y      = relu(batchnorm_train(y) * gamma + beta)
  out    = depthwise 3x3 conv of y with per-(sample,channel) filters gen

Sharding: data-parallel over batch across 8 cores (2 samples each).
BN batch statistics are merged with an in-kernel AllReduce.

Device mapping (v2 — bf16 matmul paths, collective-window overlap):
 - conv3x3 -> 18 accumulated TensorE matmuls (9 taps x 2 input-channel
   chunks) per output tile in bf16 (FWL weight loads).  Output tiles
   are 4 rows x 96 cols = 384 positions; rhs uses 2D access patterns
   into a zero-padded [98,98] bf16 input.
 - b_c1 is dropped entirely: training-mode BN subtracts the per-channel
   mean, so a constant per-channel bias cancels exactly.
 - BN stats via DVE bn_stats on each conv PSUM tile + bn_aggr + AllReduce.
 - y is spilled to DRAM in bf16 and reloaded for the BN-apply pass.
 - depthwise conv -> 9 accumulated bf16 matmuls with diagonal weight
   matrices diag(gen[:, tap]); diags + pad-zeroing + y reloads are all
   emitted before the BN scale so they run inside the AllReduce window.
 - output is written bf16 and upcast to fp32 on the host.
"""

import numpy as np
import ml_dtypes

import concourse.bass as bass
import concourse.bacc as bacc
import concourse.tile as tile
from concourse import mybir, bass_utils

F32 = mybir.dt.float32
BF16 = mybir.dt.bfloat16
U16 = mybir.dt.uint16

B, C, H, W = 16, 256, 96, 96
FS = 3
BN_EPS = 1e-5
NCORES = 8
SPC = B // NCORES          # samples per core = 2
P = 128                    # partition dim
NIC = C // P               # input channel chunks = 2
NOC = C // P               # output channel chunks = 2
HP, WP = H + 2, W + 2      # padded spatial = 98
RT = 4                     # output rows per tile
NT = H // RT               # tiles per (sample, oc) = 24
GRP = 6                    # tiles per input group (24 rows)
NG = NT // GRP             # input groups = 4
N_LOCAL = float(SPC * H * W)        # elements per (channel, core)
N_TOTAL = float(B * H * W)          # elements per channel globally

_cache = {}


def _build_program():
    nc = bacc.Bacc("TRN2", target_bir_lowering=False, debug=False,
                   num_devices=NCORES)

    cp_d = nc.dram_tensor("cp", (SPC, NIC, P, HP, WP), BF16, kind="ExternalInput")
    x_d = nc.dram_tensor("xin", (SPC, NIC, P, H, W), BF16, kind="ExternalInput")
    wT_d = nc.dram_tensor("wT", (NIC, P, 9 * NOC * P), BF16, kind="ExternalInput")
    wg_d = nc.dram_tensor("wgenT", (NIC, P, NOC * P), F32, kind="ExternalInput")
    bg_d = nc.dram_tensor("bgen", (NOC, P), F32, kind="ExternalInput")
    gam_d = nc.dram_tensor("gam", (NOC, P), F32, kind="ExternalInput")
    bet_d = nc.dram_tensor("bet", (NOC, P), F32, kind="ExternalInput")
    id_d = nc.dram_tensor("ident", (P, P), BF16, kind="ExternalInput")
    out_d = nc.dram_tensor("out", (SPC, NOC, P, H, W), BF16, kind="ExternalOutput")

    with tile.TileContext(nc) as tc:
        with (
            tc.tile_pool(name="const", bufs=1) as const,
            tc.tile_pool(name="cin", bufs=4) as cinp,
            tc.tile_pool(name="xp", bufs=2) as xp,
            tc.tile_pool(name="small", bufs=1) as small,
            tc.tile_pool(name="ybn", bufs=2) as ybnp,
            tc.tile_pool(name="yld", bufs=4) as yldp,
            tc.tile_pool(name="evac", bufs=4) as evacp,
            tc.tile_pool(name="oevac", bufs=4) as oevacp,
            tc.tile_pool(name="ps_conv", bufs=3, space="PSUM") as ps_conv,
            tc.tile_pool(name="ps_dw", bufs=3, space="PSUM") as ps_dw,
            tc.tile_pool(name="ps_gen", bufs=1, space="PSUM") as ps_gen,
            tc.tile_pool(name="dram", bufs=1, space="DRAM") as dram,
        ):
            # ---- constants / weights (emitted first, small) ----
            w_sb = const.tile([P, NIC, 9 * NOC * P], BF16)
            for ic in range(NIC):
                nc.sync.dma_start(w_sb[:, ic, :], wT_d.ap()[ic])
            wg_sb = const.tile([P, NIC, NOC * P], F32)
            for ic in range(NIC):
                nc.sync.dma_start(wg_sb[:, ic, :], wg_d.ap()[ic])
            id_sb = const.tile([P, P], BF16)
            nc.sync.dma_start(id_sb[:], id_d.ap())
            bg_sb = const.tile([P, NOC], F32)
            gam_sb = const.tile([P, NOC], F32)
            bet_sb = const.tile([P, NOC], F32)
            nc.sync.dma_start(bg_sb[:], bg_d.ap().rearrange("a p -> p a"))
            nc.sync.dma_start(gam_sb[:], gam_d.ap().rearrange("a p -> p a"))
            nc.sync.dma_start(bet_sb[:], bet_d.ap().rearrange("a p -> p a"))

            y_spill = dram.tile([SPC, NOC, P, H, W], BF16)
            ar_in_d = dram.tile([P, 2 * NOC], F32)
            ar_out_d = dram.tile([P, 2 * NOC], F32)

            stats = small.tile([P, NOC, NT * SPC * 6], F32)
            pooled = {}
            gen = {}

            def emit_pool(s, ic):
                pt = small.tile([P, 9], F32, tag=f"pooled{s}{ic}",
                                name=f"pooled{s}{ic}")
                pooled[s, ic] = pt
                for bi in range(3):
                    xblk = xp.tile([P, 32, W], BF16)
                    nc.sync.dma_start(xblk[:], x_d.ap()[s, ic, :,
                                                        32 * bi:32 * bi + 32, :])
                    for bj in range(3):
                        nc.vector.reduce_sum(
                            pt[:, bi * 3 + bj:bi * 3 + bj + 1],
                            xblk[:, :, 32 * bj:32 * bj + 32],
                            axis=mybir.AxisListType.XY)

            def emit_gen(s):
                # gen = wgenT.T @ pooled + b_gen (tiny fp32 matmuls)
                for oc in range(NOC):
                    gps = ps_gen.tile([P, 9], F32, tag="gen", bufs=2, name="gps")
                    for ic in range(NIC):
                        nc.tensor.matmul(gps[:], wg_sb[:, ic, oc * P:(oc + 1) * P],
                                         pooled[s, ic][:],
                                         start=(ic == 0), stop=(ic == NIC - 1))
                    gt = small.tile([P, 9], F32, tag=f"gen{s}{oc}",
                                    name=f"gen{s}{oc}")
                    gen[s, oc] = gt
                    nc.scalar.activation(gt[:], gps[:],
                                         mybir.ActivationFunctionType.Identity,
                                         bias=bg_sb[:, oc:oc + 1])

            def emit_conv_group(s, g):
                cin = {}
                for ic in range(NIC):
                    ct = cinp.tile([P, GRP * RT + 2, WP], BF16, name="cin")
                    cin[ic] = ct
                    nc.sync.dma_start(
                        ct[:], cp_d.ap()[s, ic, :,
                                         g * GRP * RT:(g + 1) * GRP * RT + 2, :])
                for jj in range(GRP):
                    j = g * GRP + jj
                    for oc in range(NOC):
                        ps = ps_conv.tile([P, RT, W], F32, name="ps")
                        k = 0
                        for ic in range(NIC):
                            for t in range(9):
                                dy, dx = t // 3, t % 3
                                r0 = jj * RT + dy
                                nc.tensor.matmul(
                                    ps[:],
                                    w_sb[:, ic, (t * NOC + oc) * P:
                                         (t * NOC + oc + 1) * P],
                                    cin[ic][:, r0:r0 + RT, dx:dx + W],
                                    start=(k == 0), stop=(k == 17))
                                k += 1
                        idx = (s * NT + j) * 6
                        nc.vector.bn_stats(
                            stats[:, oc, idx:idx + 6],
                            ps[:].rearrange("p a b -> p (a b)"))
                        ysb = evacp.tile([P, RT, W], BF16, name="ysb")
                        nc.scalar.copy(ysb[:], ps[:])
                        nc.sync.dma_start(
                            y_spill[s, oc, :, j * RT:(j + 1) * RT, :], ysb[:])

            # Interleave: conv groups keep PE busy from the start; x-pool
            # loads and gen matmuls slot into spare DMA/DVE capacity.
            emit_conv_group(0, 0)
            emit_pool(0, 0)
            emit_conv_group(0, 1)
            emit_pool(0, 1)
            emit_conv_group(0, 2)
            emit_pool(1, 0)
            emit_conv_group(0, 3)
            emit_pool(1, 1)
            emit_gen(0)
            emit_gen(1)
            for g in range(NG):
                emit_conv_group(1, g)

            # ---- merge stats, AllReduce ----
            ar_in = small.tile([P, 2 * NOC], F32)
            mvt = small.tile([P, NOC, 2], F32)
            tmp = small.tile([P, 4], F32)
            for oc in range(NOC):
                nc.vector.bn_aggr(mvt[:, oc, :], stats[:, oc, :])
                # sum = n * mean ; sumsq = n * (var + mean^2)
                nc.vector.tensor_scalar_mul(ar_in[:, 2 * oc:2 * oc + 1],
                                            mvt[:, oc, 0:1], N_LOCAL)
                nc.vector.tensor_mul(tmp[:, 0:1], mvt[:, oc, 0:1], mvt[:, oc, 0:1])
                nc.vector.tensor_add(tmp[:, 1:2], tmp[:, 0:1], mvt[:, oc, 1:2])
                nc.vector.tensor_scalar_mul(ar_in[:, 2 * oc + 1:2 * oc + 2],
                                            tmp[:, 1:2], N_LOCAL)
            nc.sync.dma_start(ar_in_d[:], ar_in[:])
            nc.gpsimd.collective_compute(
                "AllReduce", mybir.AluOpType.add,
                replica_groups=[list(range(NCORES))],
                ins=[ar_in_d.opt()], outs=[ar_out_d.opt()])

            # ---- work that runs inside the AllReduce window ----
            # diag(gen) weight matrices for the depthwise conv
            dgs = {}
            for s in range(SPC):
                for oc in range(NOC):
                    dg = const.tile([P, 9, P], BF16, tag=f"dg{s}{oc}",
                                    name=f"dg{s}{oc}")
                    dgs[s, oc] = dg
                    for t in range(9):
                        nc.vector.tensor_scalar_mul(dg[:, t, :], id_sb[:],
                                                    gen[s, oc][:, t:t + 1])

            # ybn tiles (bufs=2): zero the pad borders for the first two
            # (s,oc) now; reload y for the first (s,oc) from DRAM.
            RB = 24
            ybns = {}

            def emit_ybn_pad(s, oc):
                ybn = ybnp.tile([P, HP, WP], BF16, name="ybn")
                ybns[s, oc] = ybn
                nc.gpsimd.memset(ybn[:, 0, :].bitcast(U16), 0)
                nc.gpsimd.memset(ybn[:, HP - 1, :].bitcast(U16), 0)
                # interior edge pads: (r, 97) and (r+1, 0) are flat-adjacent
                pad_pairs = (ybn[:].rearrange("p a b -> p (a b)")
                             [:, WP - 1:WP - 1 + H * WP]
                             .rearrange("p (r t) -> p r t", t=WP)[:, :, 0:2])
                nc.gpsimd.memset(pad_pairs.bitcast(U16), 0)

            ylds = {}

            def emit_yld(s, oc):
                for rb in range(H // RB):
                    yld = yldp.tile([P, RB, W], BF16, name="yld")
                    ylds[s, oc, rb] = yld
                    nc.sync.dma_start(
                        yld[:], y_spill[s, oc, :, rb * RB:(rb + 1) * RB, :])

            emit_ybn_pad(0, 0)
            emit_ybn_pad(0, 1)
            emit_yld(0, 0)

            # fetch AllReduce result
            ar_out = small.tile([P, 2 * NOC], F32)
            nc.sync.dma_start(ar_out[:], ar_out_d[:])

            # ---- BN scale/bias from global stats ----
            scale = small.tile([P, NOC], F32)
            bias = small.tile([P, NOC], F32)
            w1 = small.tile([P, 8], F32)
            for oc in range(NOC):
                mu = w1[:, 0:1]
                veps = w1[:, 1:2]
                nc.vector.tensor_scalar_mul(mu, ar_out[:, 2 * oc:2 * oc + 1],
                                            1.0 / N_TOTAL)
                # var = sumsq/n - mu^2 ; veps = var + eps
                nc.vector.tensor_scalar_mul(w1[:, 2:3],
                                            ar_out[:, 2 * oc + 1:2 * oc + 2],
                                            1.0 / N_TOTAL)
                nc.vector.tensor_mul(w1[:, 3:4], mu, mu)
                nc.vector.tensor_sub(w1[:, 4:5], w1[:, 2:3], w1[:, 3:4])
                nc.vector.tensor_scalar_add(veps, w1[:, 4:5], BN_EPS)
                # r = rsqrt(veps): reciprocal + ACT sqrt + one Newton step
                inv = w1[:, 5:6]
                nc.vector.reciprocal(inv, veps)
                r = w1[:, 6:7]
                nc.scalar.activation(r, inv, mybir.ActivationFunctionType.Sqrt)
                # r <- 0.5 * r * (3 - veps * r^2)
                nc.vector.tensor_mul(w1[:, 7:8], r, r)
                nc.vector.tensor_mul(w1[:, 7:8], w1[:, 7:8], veps)
                nc.vector.tensor_scalar(w1[:, 7:8], w1[:, 7:8], -0.5, 1.5,
                                        op0=mybir.AluOpType.mult,
                                        op1=mybir.AluOpType.add)
                nc.vector.tensor_mul(r, r, w1[:, 7:8])
                # scale = gamma * r ; bias = beta - mu * scale
                nc.vector.tensor_mul(scale[:, oc:oc + 1], gam_sb[:, oc:oc + 1], r)
                nc.vector.tensor_mul(w1[:, 7:8], mu, scale[:, oc:oc + 1])
                nc.vector.tensor_sub(bias[:, oc:oc + 1], bet_sb[:, oc:oc + 1],
                                     w1[:, 7:8])

            # ---- BN apply + ReLU + dynamic depthwise conv ----
            order = [(s, oc) for s in range(SPC) for oc in range(NOC)]
            for i, (s, oc) in enumerate(order):
                ybn = ybns[s, oc]
                for rb in range(H // RB):
                    yld = ylds[s, oc, rb]
                    nc.scalar.activation(
                        ybn[:, 1 + rb * RB:1 + (rb + 1) * RB, 1:W + 1],
                        yld[:], mybir.ActivationFunctionType.Relu,
                        bias=bias[:, oc:oc + 1], scale=scale[:, oc:oc + 1])
                # prepare the next (s,oc): pad borders + y reload now so
                # they overlap this iteration's matmuls
                if i + 1 < len(order):
                    sn, ocn = order[i + 1]
                    if (sn, ocn) not in ybns:
                        emit_ybn_pad(sn, ocn)
                    emit_yld(sn, ocn)
                dg = dgs[s, oc]
                for j in range(NT):
                    pd = ps_dw.tile([P, RT, W], F32, name="pd")
                    for t in range(9):
                        dy, dx = t // 3, t % 3
                        nc.tensor.matmul(
                            pd[:], dg[:, t, :],
                            ybn[:, j * RT + dy:j * RT + dy + RT, dx:dx + W],
                            start=(t == 0), stop=(t == 8))
                    osb = oevacp.tile([P, RT, W], BF16, name="osb")
                    nc.vector.tensor_copy(osb[:], pd[:])
                    nc.sync.dma_start(
                        out_d.ap()[s, oc, :, j * RT:(j + 1) * RT, :], osb[:])

    nc.compile()
    return nc


def _prep_inputs(x, convoluted, w_gen, b_gen, w_c1, b_c1, gamma, beta):
    bf16 = ml_dtypes.bfloat16
    x = np.asarray(x, dtype=np.float32)
    convoluted = np.asarray(convoluted, dtype=np.float32)
    w_gen = np.asarray(w_gen, dtype=np.float32)
    b_gen = np.asarray(b_gen, dtype=np.float32)
    w_c1 = np.asarray(w_c1, dtype=np.float32)
    gamma = np.asarray(gamma, dtype=np.float32)
    beta = np.asarray(beta, dtype=np.float32)

    cp = np.zeros((B, NIC, P, HP, WP), bf16)
    cp[:, :, :, 1:H + 1, 1:W + 1] = convoluted.reshape(B, NIC, P, H, W)
    xr = np.ascontiguousarray(x.reshape(B, NIC, P, H, W).astype(bf16))
    # wT[ic, i, ((t*NOC)+oc)*P+o] = w_c1[oc*P+o, ic*P+i, dy, dx]
    wT = np.ascontiguousarray(
        w_c1.reshape(NOC, P, NIC, P, 9).transpose(2, 3, 4, 0, 1)
    ).reshape(NIC, P, 9 * NOC * P).astype(bf16)
    # wgenT[ic, c, oc*P+o] = w_gen[oc*P+o, ic*P+c] / 1024  (pool mean divisor)
    wgT = np.ascontiguousarray(
        (w_gen[:, :, 0, 0] / 1024.0).reshape(NOC, P, NIC, P).transpose(2, 3, 0, 1)
    ).reshape(NIC, P, NOC * P)
    shared = {
        "wT": wT, "wgenT": wgT,
        "bgen": np.ascontiguousarray(b_gen.reshape(NOC, P)),
        "gam": np.ascontiguousarray(gamma.reshape(NOC, P)),
        "bet": np.ascontiguousarray(beta.reshape(NOC, P)),
        "ident": np.eye(P, dtype=np.float32).astype(bf16),
    }
    in_maps = []
    for k in range(NCORES):
        m = dict(shared)
        m["cp"] = np.ascontiguousarray(cp[k * SPC:(k + 1) * SPC])
        m["xin"] = np.ascontiguousarray(xr[k * SPC:(k + 1) * SPC])
        in_maps.append(m)
    return in_maps


def _run(inputs, trace=False):
    if "nc" not in _cache:
        _cache["nc"] = _build_program()
    nc = _cache["nc"]
    in_maps = _prep_inputs(**inputs)
    res = bass_utils.run_bass_kernel_spmd(
        nc, in_maps, core_ids=list(range(NCORES)), trace=trace)
    outs = [r["out"].astype(np.float32).reshape(SPC, C, H, W)
            for r in res.results]
    full = np.concatenate(outs, axis=0)
    return full, res


def kernel(**inputs) -> np.ndarray:
    out, _ = _run(inputs, trace=False)
    return out
